# revision 7
# baseline (speedup 1.0000x reference)
"""AtomwiseLinear 3-expert MoE routing kernel for 8 TRN2 NeuronCores.

Strategy (data-parallel over atoms, per sharding hint):
  - Each core gets 125,000 atoms, padded to 126,720 = 33 chunks x 3840 atoms.
  - Per chunk: SWDGE DMA loads x (f32 in HBM) casting to bf16 in SBUF with a
    p-major layout (partition p holds G=30 consecutive atom rows).
  - PE transposes each [128 atom x 128 feat] square (bf16, 1 cyc/row) into
    PSUM, DVE copies batches of 5 squares back to SBUF, then PE computes
    out[a, 96] = xT.T @ Wcat for all 3 experts in one matmul (bf16, f32 accum).
  - Expert selection: out = P0; copy_predicated(out, ids, P1);
    copy_predicated(out, max(ids-1,0), P2) — masks broadcast along the
    32-wide output via zero-stride APs. Output DMA'd back as f32.
HBM traffic stays the honest f32 644 MB total; compute runs bf16.
"""

import sys

sys.path.insert(0, "/opt/trn_rl_repo")

import numpy as np
import ml_dtypes

import concourse.mybir as mybir
import concourse.bacc as bacc
import concourse.tile as tile
from concourse.bass_utils import run_bass_kernel_spmd

N_CORES = 8
N_ATOMS = 1_000_000
F_IN = 128
F_OUT = 32
P = 128
HALF = 15                   # squares per compute group (one 3-bank PSUM tile)
CHUNK_SQ = [120] * 8 + [17]  # squares per DMA chunk (8 big + tail)
SQ_TOTAL = sum(CHUNK_SQ)     # 977
NPAD = P * SQ_TOTAL          # 125056 >= 125000
NCORE = N_ATOMS // N_CORES   # 125000

bf16 = mybir.dt.bfloat16
f32 = mybir.dt.float32
i32 = mybir.dt.int32

_NC_CACHE = {}


def build_bass(repeat=0):
    """repeat=0: production kernel. repeat=R>0: wraps the whole body in an
    on-device For_i loop running it R times (timing-only variant — lets a
    differential wall-clock measurement cancel host/RPC dispatch overhead)."""
    key = ("nc", repeat)
    if key in _NC_CACHE:
        return _NC_CACHE[key]
    nc = bacc.Bacc("TRN2", target_bir_lowering=False, debug=False,
                   num_devices=N_CORES)
    x_d = nc.dram_tensor("x", (NPAD, F_IN), f32, kind="ExternalInput")
    ids_d = nc.dram_tensor("expert_ids", (NPAD,), i32, kind="ExternalInput")
    w_d = nc.dram_tensor("wcat", (F_IN, 3 * F_OUT), f32, kind="ExternalInput")
    out_d = nc.dram_tensor("out", (NPAD, F_OUT), f32, kind="ExternalOutput")
    ident = nc.inline_tensor(np.eye(P, dtype=ml_dtypes.bfloat16), name="ident")

    with tile.TileContext(nc) as tc:
        with (
            tc.tile_pool(name="const", bufs=1) as cpool,
            tc.tile_pool(name="xin", bufs=2) as xpool,
            tc.tile_pool(name="xt", bufs=4) as xtpool,
            tc.tile_pool(name="outp", bufs=2) as opool,
            tc.tile_pool(name="pT", bufs=2, space="PSUM") as ptpool,
            tc.tile_pool(name="py", bufs=2, space="PSUM") as pypool,
        ):
            ident_sb = cpool.tile([P, P], bf16)
            nc.sync.dma_start(ident_sb[:], ident.ap())
            w_sb = cpool.tile([F_IN, 3 * F_OUT], bf16)
            nc.gpsimd.dma_start(w_sb[:], w_d.ap())  # f32 -> bf16 cast

            import contextlib
            loop_ctx = (tc.For_i(0, repeat, 1) if repeat
                        else contextlib.nullcontext())
            with loop_ctx:
                _body(nc, tc, x_d, ids_d, out_d, cpool, xpool, xtpool, opool,
                      ptpool, pypool, ident_sb, w_sb)
    nc.compile()
    _NC_CACHE[key] = nc
    return nc


def _select(nc, py, ob, ids_sb, m2_sb, o0, i0, nb, ns):
    """Write ob[:, o0 : o0+nb*ns*F_OUT] = expert-selected outputs of the
    nb*ns squares whose matmul results sit in py[:, 0:nb, s*96:(s+1)*96] and
    whose ids are ids_sb[:, i0 : i0+nb*ns]."""
    w = 96 * ns
    pv = py[:, 0:nb, 0:w].rearrange("p b (s j) -> p b s j", j=96)
    ov = ob[:, o0:o0 + nb * ns * F_OUT].rearrange(
        "p (b s j) -> p b s j", b=nb, s=ns, j=F_OUT)
    idv = ids_sb[:, i0:i0 + nb * ns].rearrange(
        "p (b s) -> p b s", b=nb)[:, :, :, None].broadcast_to(
        [P, nb, ns, F_OUT])
    m2v = m2_sb[:, i0:i0 + nb * ns].rearrange(
        "p (b s) -> p b s", b=nb)[:, :, :, None].broadcast_to(
        [P, nb, ns, F_OUT])
    nc.vector.tensor_copy(ov, pv[:, :, :, 0:F_OUT])
    nc.vector.copy_predicated(ov, idv, pv[:, :, :, F_OUT:2 * F_OUT])
    nc.vector.copy_predicated(ov, m2v, pv[:, :, :, 2 * F_OUT:3 * F_OUT])


def _body(nc, tc, x_d, ids_d, out_d, cpool, xpool, xtpool, opool,
          ptpool, pypool, ident_sb, w_sb):
    a0 = 0  # first atom row of current chunk
    for c, nsq in enumerate(CHUNK_SQ):
        catoms = P * nsq
        xc = xpool.tile([P, nsq * F_IN], bf16, tag="xc")
        x_ap = x_d.ap()[a0:a0 + catoms, :].rearrange(
            "(p g) f -> p (g f)", p=P)
        nc.gpsimd.dma_start(xc[:], x_ap)  # f32 -> bf16 cast
        ids_sb = opool.tile([P, nsq], i32, tag="ids")
        nc.sync.dma_start(
            ids_sb[:], ids_d.ap()[a0:a0 + catoms].rearrange("(p g) -> p g", p=P))
        m2_sb = opool.tile([P, nsq], i32, tag="m2")
        nc.vector.tensor_scalar(
            m2_sb[:], ids_sb[:], 1, 0,
            op0=mybir.AluOpType.subtract, op1=mybir.AluOpType.max)
        ob = opool.tile([P, nsq * F_OUT], f32, tag="ob")

        for g0 in range(0, nsq, HALF):  # compute groups of <=15 squares
            ng = min(HALF, nsq - g0)
            nbank = (ng + 4) // 5
            py = pypool.tile([P, 3, 512], f32, tag="py")  # 3 PSUM banks
            for grp0 in range(0, ng, 5):  # sub-groups of <=5 squares
                npt = min(5, ng - grp0)
                pt = ptpool.tile([P, 5 * P], bf16, tag="pt")
                for k in range(npt):
                    g = g0 + grp0 + k
                    nc.tensor.transpose(
                        pt[:, k * P:(k + 1) * P],
                        xc[:, g * F_IN:(g + 1) * F_IN],
                        ident_sb[:])
                xt = xtpool.tile([P, 5 * P], bf16, tag="xt")
                nc.vector.tensor_copy(xt[:, 0:npt * P], pt[:, 0:npt * P])
                for k in range(npt):
                    b, s = divmod(grp0 + k, 5)
                    nc.tensor.matmul(
                        py[:, b, s * 96:s * 96 + 96],
                        xt[:, k * P:(k + 1) * P],
                        w_sb[:], start=True, stop=True)
            # expert-select this group: full banks first, then remainder
            nb_full, rem = divmod(ng, 5)
            if nb_full:
                _select(nc, py, ob, ids_sb, m2_sb,
                        o0=g0 * F_OUT, i0=g0, nb=nb_full, ns=5)
            if rem:
                _select(nc, py[:, nb_full:3, :], ob, ids_sb, m2_sb,
                        o0=(g0 + nb_full * 5) * F_OUT, i0=g0 + nb_full * 5,
                        nb=1, ns=rem)
        o_ap = out_d.ap()[a0:a0 + catoms, :].rearrange(
            "(p g) f -> p (g f)", p=P)
        nc.sync.dma_start(o_ap, ob[:])
        a0 += catoms


def make_in_maps(x, W1, W2, W3, expert_ids):
    x = np.ascontiguousarray(np.asarray(x, dtype=np.float32))
    ids = np.ascontiguousarray(np.asarray(expert_ids, dtype=np.int32))
    wcat = np.concatenate(
        [np.asarray(W1, np.float32), np.asarray(W2, np.float32),
         np.asarray(W3, np.float32)], axis=1)
    wcat = np.ascontiguousarray(wcat)
    in_maps = []
    for c in range(N_CORES):
        xs = np.zeros((NPAD, F_IN), np.float32)
        xs[:NCORE] = x[c * NCORE:(c + 1) * NCORE]
        isd = np.zeros((NPAD,), np.int32)
        isd[:NCORE] = ids[c * NCORE:(c + 1) * NCORE]
        in_maps.append({"x": xs, "expert_ids": isd, "wcat": wcat})
    return in_maps


def kernel(x, W1, W2, W3, expert_ids):
    nc = build_bass()
    in_maps = make_in_maps(x, W1, W2, W3, expert_ids)
    res = run_bass_kernel_spmd(nc, in_maps, core_ids=list(range(N_CORES)))
    out = np.concatenate(
        [res.results[c]["out"][:NCORE] for c in range(N_CORES)], axis=0)
    return np.ascontiguousarray(out.astype(np.float32, copy=False))


if __name__ == "__main__":
    rng = np.random.default_rng(0)
    x = rng.standard_normal((N_ATOMS, F_IN)).astype(np.float32)
    ids = rng.integers(0, 3, N_ATOMS).astype(np.int32)
    sc = 1.0 / np.sqrt(F_IN)
    W1, W2, W3 = (rng.standard_normal((F_IN, F_OUT)).astype(np.float32) * sc
                  for _ in range(3))
    out = kernel(x, W1, W2, W3, ids)
    exact = np.stack([x @ W1, x @ W2, x @ W3])[ids, np.arange(N_ATOMS)]
    rel = np.linalg.norm(out - exact) / np.linalg.norm(exact)
    print("rel err vs exact f32:", rel)


# revision 23
# speedup vs baseline: 1.2342x; 1.2342x over previous
"""AtomwiseLinear 3-expert MoE routing kernel for 8 TRN2 NeuronCores.

Strategy (data-parallel over atoms, per sharding hint):
  - Each core gets 125,000 atoms, padded to 125,056 = 977 squares of 128;
    DMA chunks of 120 squares (7.9 MB), each split into 4 sub-DMAs for
    fine-grained completion, p-major layout (partition p holds consecutive
    atom rows). SWDGE DMA casts f32 (HBM) -> bf16 (SBUF) inline.
  - PE transposes each [128 atom x 128 feat] square (bf16 transpose-mode
    matmul vs identity) into PSUM in groups of 8 per bank; the scalar engine
    (ACT) copies each group back to SBUF; PE then computes
    out[a, 96] = xT.T @ Wcat for all 3 experts in one bf16 matmul per square
    (f32 PSUM accumulate), 15 squares per 3-bank PSUM tile.
  - Expert selection on DVE: out = P0; copy_predicated(out, ids, P1);
    copy_predicated(out, max(ids-1,0), P2) — int masks broadcast along the
    32-wide output via zero-stride APs, one pass per 15-square group.
  - Output stored f16 (on-device downcast, ~5e-4 extra rel err), host
    upcasts to f32. HBM traffic: 512 MB x (f32) + 4 MB ids + 64 MB out.
Measured ~230 us/core on 8 TRN2 NeuronCores (~2.9 TB/s aggregate, at the
per-core HBM bandwidth ceiling); rel err vs f32 reference ~2.4e-3.
"""

import sys

sys.path.insert(0, "/opt/trn_rl_repo")

import numpy as np
import ml_dtypes

import concourse.mybir as mybir
import concourse.bacc as bacc
import concourse.tile as tile
from concourse.bass_utils import run_bass_kernel_spmd

N_CORES = 8
N_ATOMS = 1_000_000
F_IN = 128
F_OUT = 32
P = 128
PYG = 15                    # squares per PSUM-y tile (3 banks)
XSPLIT = 4                  # input sub-DMAs per chunk
CHUNK_SQ = [120] * 8 + [17]  # squares per DMA chunk (8 big + tail)
SQ_TOTAL = sum(CHUNK_SQ)     # 977
NPAD = P * SQ_TOTAL          # 125056 >= 125000
NCORE = N_ATOMS // N_CORES   # 125000

bf16 = mybir.dt.bfloat16
f32 = mybir.dt.float32
i32 = mybir.dt.int32

_NC_CACHE = {}


def build_bass(repeat=0, variant="full", act_every=1, out_f16=True):
    """repeat=0: production kernel. repeat=R>0: wraps the whole body in an
    on-device For_i loop running it R times (timing-only variant — lets a
    differential wall-clock measurement cancel host/RPC dispatch overhead).
    variant: "full" | "dma" (DMAs only) | "compute" (no x/out DMAs)."""
    key = ("nc", repeat, variant, act_every, out_f16)
    if key in _NC_CACHE:
        return _NC_CACHE[key]
    nc = bacc.Bacc("TRN2", target_bir_lowering=False, debug=False,
                   num_devices=N_CORES)
    x_d = nc.dram_tensor("x", (NPAD, F_IN), f32, kind="ExternalInput")
    ids_d = nc.dram_tensor("expert_ids", (NPAD,), i32, kind="ExternalInput")
    w_d = nc.dram_tensor("wcat", (F_IN, 3 * F_OUT), f32, kind="ExternalInput")
    odt = mybir.dt.float16 if out_f16 else f32
    out_d = nc.dram_tensor("out", (NPAD, F_OUT), odt, kind="ExternalOutput")
    ident = nc.inline_tensor(np.eye(P, dtype=ml_dtypes.bfloat16), name="ident")

    with tile.TileContext(nc) as tc:
        with (
            tc.tile_pool(name="const", bufs=1) as cpool,
            tc.tile_pool(name="xin", bufs=(2 if variant == "dmaf32" else 4)) as xpool,
            tc.tile_pool(name="xt", bufs=4) as xtpool,
            tc.tile_pool(name="outp", bufs=2) as opool,
            tc.tile_pool(name="pT", bufs=2, space="PSUM") as ptpool,
            tc.tile_pool(name="py", bufs=2, space="PSUM") as pypool,
        ):
            ident_sb = cpool.tile([P, P], bf16)
            nc.sync.dma_start(ident_sb[:], ident.ap())
            w_sb = cpool.tile([F_IN, 3 * F_OUT], bf16)
            nc.gpsimd.dma_start(w_sb[:], w_d.ap())  # f32 -> bf16 cast

            import contextlib
            _hint = (mybir.EngineType.PE, mybir.EngineType.DVE,
                     mybir.EngineType.Activation, mybir.EngineType.SP,
                     mybir.EngineType.Pool)
            loop_ctx = (tc.For_i(0, repeat, 1, hint_engines=_hint) if repeat
                        else contextlib.nullcontext())
            with loop_ctx:
                _body(nc, tc, x_d, ids_d, out_d, cpool, xpool, xtpool, opool,
                      ptpool, pypool, ident_sb, w_sb, variant, act_every, odt)
    nc.compile()
    _NC_CACHE[key] = nc
    return nc


def _select(nc, py, ob, ids_sb, m2_sb, o0, i0, nb, ns):
    """Write ob[:, o0 : o0+nb*ns*F_OUT] = expert-selected outputs of the
    nb*ns squares whose matmul results sit in py[:, 0:nb, s*96:(s+1)*96] and
    whose ids are ids_sb[:, i0 : i0+nb*ns]."""
    w = 96 * ns
    pv = py[:, 0:nb, 0:w].rearrange("p b (s j) -> p b s j", j=96)
    ov = ob[:, o0:o0 + nb * ns * F_OUT].rearrange(
        "p (b s j) -> p b s j", b=nb, s=ns, j=F_OUT)
    idv = ids_sb[:, i0:i0 + nb * ns].rearrange(
        "p (b s) -> p b s", b=nb)[:, :, :, None].broadcast_to(
        [P, nb, ns, F_OUT])
    m2v = m2_sb[:, i0:i0 + nb * ns].rearrange(
        "p (b s) -> p b s", b=nb)[:, :, :, None].broadcast_to(
        [P, nb, ns, F_OUT])
    nc.vector.tensor_copy(ov, pv[:, :, :, 0:F_OUT])
    nc.vector.copy_predicated(ov, idv, pv[:, :, :, F_OUT:2 * F_OUT])
    nc.vector.copy_predicated(ov, m2v, pv[:, :, :, 2 * F_OUT:3 * F_OUT])


def _copy(nc, eng, out, in_):
    if eng is nc.scalar:
        nc.scalar.activation(out, in_, mybir.ActivationFunctionType.Copy)
    else:
        nc.vector.tensor_copy(out, in_)


def _body(nc, tc, x_d, ids_d, out_d, cpool, xpool, xtpool, opool,
          ptpool, pypool, ident_sb, w_sb, variant="full", act_every=0,
          odt=f32):
    do_dma = variant in ("full", "dma", "dmaf32", "dmain")
    do_compute = variant in ("full", "compute")
    no_cast = variant == "dmaf32"
    no_out = variant == "dmain"
    if not do_dma:
        # compute-only: one shared garbage input, written once
        xc_shared = cpool.tile([P, max(CHUNK_SQ) * F_IN], bf16)
        nc.vector.memset(xc_shared[:], 0.25)
        ids_shared = cpool.tile([P, max(CHUNK_SQ)], i32)
        nc.vector.memset(ids_shared[:], 1)
    a0 = 0  # first atom row of current chunk
    for c, nsq in enumerate(CHUNK_SQ):
        catoms = P * nsq
        if do_dma:
            x_ap = x_d.ap()[a0:a0 + catoms, :].rearrange(
                "(p g) f -> p (g f)", p=P)
            if no_cast:
                xc = xpool.tile([P, nsq * F_IN], f32, tag="xc")
                nc.sync.dma_start(xc[:], x_ap)
            else:
                xc = xpool.tile([P, nsq * F_IN], bf16, tag="xc")
                # split into sub-DMAs: compute on the first slice can start
                # as soon as it lands (finer completion granularity)
                hsq = max(1, (nsq + XSPLIT - 1) // XSPLIT)
                for q0 in range(0, nsq, hsq):
                    q1 = min(q0 + hsq, nsq)
                    nc.gpsimd.dma_start(xc[:, q0 * F_IN:q1 * F_IN],
                                        x_ap[:, q0 * F_IN:q1 * F_IN])
            ids_sb = opool.tile([P, nsq], i32, tag="ids")
            nc.sync.dma_start(
                ids_sb[:],
                ids_d.ap()[a0:a0 + catoms].rearrange("(p g) -> p g", p=P))
        else:
            xc = xc_shared[:, 0:nsq * F_IN]
            ids_sb = ids_shared[:, 0:nsq]
        m2_sb = opool.tile([P, nsq], i32, tag="m2")
        if do_compute:
            nc.vector.tensor_scalar(
                m2_sb[:], ids_sb[:], 1, 0,
                op0=mybir.AluOpType.subtract, op1=mybir.AluOpType.max)
        ob = opool.tile([P, nsq * F_OUT], odt, tag="ob")
        if not do_compute:
            nc.vector.memset(ob[:, 0:1], 0.0)

        if do_compute:
            # transpose squares in groups of 8 (one full PSUM bank per group),
            # PSUM-y tiles hold 15 squares (3 banks); both pools double buffered
            TG = 8
            xts = {}  # transpose-group index -> xt sbuf tile
            for t0 in range(0, nsq, TG):
                nt = min(TG, nsq - t0)
                pt = ptpool.tile([P, TG * P], bf16, tag="pt")
                for k in range(nt):
                    g = t0 + k
                    nc.tensor.transpose(
                        pt[:, k * P:(k + 1) * P],
                        xc[:, g * F_IN:(g + 1) * F_IN],
                        ident_sb[:])
                xt = xtpool.tile([P, TG * P], bf16, tag="xt")
                eng = (nc.scalar if act_every and (t0 // TG) % act_every == 0
                       else nc.vector)
                _copy(nc, eng, xt[:, 0:nt * P], pt[:, 0:nt * P])
                xts[t0 // TG] = xt
            for g0 in range(0, nsq, PYG):  # psum-y groups of <=10 squares
                ng = min(PYG, nsq - g0)
                py = pypool.tile([P, 3, 512], f32, tag="py")  # 3 PSUM banks
                for k in range(ng):
                    g = g0 + k
                    xt = xts[g // TG]
                    b, s = divmod(k, 5)
                    nc.tensor.matmul(
                        py[:, b, s * 96:s * 96 + 96],
                        xt[:, (g % TG) * P:(g % TG + 1) * P],
                        w_sb[:], start=True, stop=True)
                # expert-select this group: full banks first, then remainder
                nb_full, rem = divmod(ng, 5)
                if nb_full:
                    _select(nc, py, ob, ids_sb, m2_sb,
                            o0=g0 * F_OUT, i0=g0, nb=nb_full, ns=5)
                if rem:
                    _select(nc, py[:, nb_full:3, :], ob, ids_sb, m2_sb,
                            o0=(g0 + nb_full * 5) * F_OUT, i0=g0 + nb_full * 5,
                            nb=1, ns=rem)
        if do_dma and not no_out:
            o_ap = out_d.ap()[a0:a0 + catoms, :].rearrange(
                "(p g) f -> p (g f)", p=P)
            nc.sync.dma_start(o_ap, ob[:])
        a0 += catoms


def make_in_maps(x, W1, W2, W3, expert_ids):
    x = np.ascontiguousarray(np.asarray(x, dtype=np.float32))
    ids = np.ascontiguousarray(np.asarray(expert_ids, dtype=np.int32))
    wcat = np.concatenate(
        [np.asarray(W1, np.float32), np.asarray(W2, np.float32),
         np.asarray(W3, np.float32)], axis=1)
    wcat = np.ascontiguousarray(wcat)
    in_maps = []
    for c in range(N_CORES):
        xs = np.zeros((NPAD, F_IN), np.float32)
        xs[:NCORE] = x[c * NCORE:(c + 1) * NCORE]
        isd = np.zeros((NPAD,), np.int32)
        isd[:NCORE] = ids[c * NCORE:(c + 1) * NCORE]
        in_maps.append({"x": xs, "expert_ids": isd, "wcat": wcat})
    return in_maps


def kernel(x, W1, W2, W3, expert_ids):
    nc = build_bass()
    in_maps = make_in_maps(x, W1, W2, W3, expert_ids)
    res = run_bass_kernel_spmd(nc, in_maps, core_ids=list(range(N_CORES)))
    out = np.concatenate(
        [res.results[c]["out"][:NCORE].astype(np.float32)
         for c in range(N_CORES)], axis=0)
    return np.ascontiguousarray(out)


if __name__ == "__main__":
    rng = np.random.default_rng(0)
    x = rng.standard_normal((N_ATOMS, F_IN)).astype(np.float32)
    ids = rng.integers(0, 3, N_ATOMS).astype(np.int32)
    sc = 1.0 / np.sqrt(F_IN)
    W1, W2, W3 = (rng.standard_normal((F_IN, F_OUT)).astype(np.float32) * sc
                  for _ in range(3))
    out = kernel(x, W1, W2, W3, ids)
    exact = np.stack([x @ W1, x @ W2, x @ W3])[ids, np.arange(N_ATOMS)]
    rel = np.linalg.norm(out - exact) / np.linalg.norm(exact)
    print("rel err vs exact f32:", rel)
